# revision 9
# baseline (speedup 1.0000x reference)
"""GCN encoder (VGAE-style, 6 GCNConv) on 8 trn2 NeuronCores.

Strategy: partition nodes (and their aggregation work) across the 8 cores;
weights replicated. Per layer: each core computes table rows
h'[n] = dis[n] * (act[n] @ W) for its own 6250 nodes, an AllGather assembles
the full 50000x128 table on every core, then each core aggregates its own
edges (sorted by dst, grouped into 128-dst-node groups) with dma_gather +
one-hot indicator matmuls accumulating segment sums in PSUM.
norm factorization: norm[e] = dis[src]*dis[dst] is applied as a row scale on
the table (src side) and a per-partition scale at PSUM evacuation (dst side).
"""
import numpy as np

import concourse.bass as bass
import concourse.mybir as mybir
import concourse.tile as tile
import concourse.bacc as bacc
from concourse.bass_utils import run_bass_kernel_spmd

P = 128
NCORES = 8
N = 50000
E = 1600000
D = 128           # IN_C == HID == 128
OUTC = 64
NPC = N // NCORES          # 6250 nodes per core
G = (NPC + P - 1) // P     # 49 groups per core
LAST_ROWS = NPC - (G - 1) * P  # 106
SPLIT = 32768              # int16 index limit for dma_gather


def _set_dims(n, e):
    """Test hook: shrink the problem (n must be divisible by NCORES)."""
    global N, E, NPC, G, LAST_ROWS
    N, E = n, e
    NPC = N // NCORES
    G = (NPC + P - 1) // P
    LAST_ROWS = NPC - (G - 1) * P
SELU_L = 1.0507009873554805
SELU_A = 1.6732632423543772

f32 = mybir.dt.float32
i16 = mybir.dt.int16

_CACHE = {}


# ----------------------------------------------------------------- host prep
def _pack_idx16(vals, ntiles):
    """flat edge order i -> [128, ntiles*8] int16; i at (i%16, i//16), the
    16-row block replicated to all 8 gpsimd cores."""
    n = ntiles * P
    blk = np.zeros((16, n // 16), np.int16)
    if len(vals):
        i = np.arange(len(vals))
        blk[i % 16, i // 16] = vals.astype(np.int16)
    return np.tile(blk, (8, 1))


def _preprocess(edge_index):
    src = np.asarray(edge_index[0], dtype=np.int64)
    dst = np.asarray(edge_index[1], dtype=np.int64)
    # self loops
    loops = np.arange(N, dtype=np.int64)
    src = np.concatenate([src, loops])
    dst = np.concatenate([dst, loops])

    deg = np.bincount(dst, minlength=N).astype(np.float32)  # includes self loop
    dis = 1.0 / np.sqrt(deg)

    core = dst // NPC
    g = (dst - core * NPC) // P
    gid = core * G + g
    segB = (src >= SPLIT).astype(np.int64)
    key = gid * 2 + segB
    order = np.argsort(key, kind="stable")
    src_s, dst_s, key_s = src[order], dst[order], key[order]
    counts = np.bincount(key, minlength=NCORES * G * 2)
    starts = np.concatenate([[0], np.cumsum(counts)[:-1]])

    nA = counts[0::2].reshape(NCORES, G)
    nB = counts[1::2].reshape(NCORES, G)
    TA = int(np.ceil(nA.max() / P))
    TB = int(np.ceil(nB.max() / P))
    TT = TA + TB

    per_core = []
    for k in range(NCORES):
        idxA = np.zeros((P, G * TA * 8), np.int16)
        idxB = np.zeros((P, G * TB * 8), np.int16)
        dstloc = np.full((P, G * TT), 500.0, np.float32)
        for gg in range(G):
            base = k * NPC + gg * P
            for seg, (T, idxarr, coloff) in enumerate(
                    [(TA, idxA, 0), (TB, idxB, TA)]):
                kk = (k * G + gg) * 2 + seg
                s0, cnt = starts[kk], counts[kk]
                sv = src_s[s0:s0 + cnt] - (SPLIT if seg else 0)
                dv = dst_s[s0:s0 + cnt] - base
                flat = np.zeros(T * P, np.int64)
                flat[:cnt] = sv
                idxarr[:, gg * T * 8:(gg + 1) * T * 8] = _pack_idx16(flat, T)
                i = np.arange(cnt)
                dstloc[i % P, gg * TT + coloff + i // P] = dv
        dis_k = np.zeros((P, G), np.float32)
        dcol = dis[k * NPC:(k + 1) * NPC]
        dis_k.T.flat[:NPC] = dcol          # dis_k[p, g] = dis[k*NPC + g*128 + p]
        per_core.append(dict(idxA=idxA, idxB=idxB, dstloc=dstloc, dis=dis_k))
    return TA, TB, per_core


# ------------------------------------------------------------ device program
def _build(TA, TB, use_bias, sim_safe=False, n_passes=5, skip_own=False):
    TT = TA + TB
    nc = bacc.Bacc("TRN2", target_bir_lowering=False, debug=False,
                   enable_asserts=False, num_devices=NCORES)

    def inp(name, shape, dt=f32):
        return nc.dram_tensor(name, shape, dt, kind="ExternalInput")

    idxA_in = inp("idxA", [P, G * TA * 8], i16)
    idxB_in = inp("idxB", [P, G * TB * 8], i16) if TB else None
    dstloc_in = inp("dstloc", [P, G * TT])
    iota_in = inp("iota", [P, P])
    ident_in = inp("ident", [P, P])
    dis_in = inp("dis_sc", [P, G])
    dis_lam_in = inp("dis_lam", [P, G])
    dis_neg_in = inp("dis_neg", [P, G])
    xT_in = inp("xT", [P, G * P])
    w_in = [inp(f"W{i}", [P, P]) for i in range(5)]  # W0..W3, Wmulv
    bb_in = [inp(f"BB{i}", [P, P]) for i in range(5)] if use_bias else []

    mu_out = nc.dram_tensor("mu_out", [NPC, OUTC], f32, kind="ExternalOutput")
    lv_out = nc.dram_tensor("lv_out", [NPC, OUTC], f32, kind="ExternalOutput")

    h_own = nc.dram_tensor("h_own", [NPC, D], f32)
    tables = [nc.dram_tensor(f"table{i}", [N, D], f32, addr_space="Shared")
              for i in range(2)]

    RG = [list(range(NCORES))]
    AF = mybir.ActivationFunctionType

    with tile.TileContext(nc) as tc:
        with (
            tc.tile_pool(name="const", bufs=1) as cpool,
            tc.tile_pool(name="msg", bufs=3) as msg_pool,
            tc.tile_pool(name="ind", bufs=4) as ind_pool,
            tc.tile_pool(name="act", bufs=3) as act_pool,
            tc.tile_pool(name="tmp", bufs=4) as tmp_pool,
            tc.tile_pool(name="hps", bufs=3, space="PSUM") as agg_psum,
            tc.tile_pool(name="tps", bufs=2, space="PSUM") as tr_psum,
            tc.tile_pool(name="zps", bufs=2, space="PSUM") as z_psum,
        ):
            def load(ap_in, shape, tag, dt=f32):
                t = cpool.tile(shape, dt, tag=tag)
                nc.sync.dma_start(out=t[:], in_=ap_in[:, :])
                return t

            idxA = load(idxA_in, [P, G * TA * 8], "c_idxA", i16)
            idxB = (load(idxB_in, [P, G * TB * 8], "c_idxB", i16)
                    if TB else None)
            dstloc = load(dstloc_in, [P, G * TT], "c_dstloc")
            iota = load(iota_in, [P, P], "c_iota")
            ident = load(ident_in, [P, P], "c_ident")
            dis_sc = load(dis_in, [P, G], "c_dis")
            dis_lam = load(dis_lam_in, [P, G], "c_dlam")
            dis_neg = load(dis_neg_in, [P, G], "c_dneg")
            xT = load(xT_in, [P, G * P], "c_xT")
            W = [load(w, [P, P], f"c_W{i}") for i, w in enumerate(w_in)]
            BB = ([load(b, [P, P], f"c_BB{i}") for i, b in enumerate(bb_in)]
                  if use_bias else None)

            def own_rows(g, z_ps, h_dram):
                """scale z (PSUM [128 nodes, D]) by dis and store node rows."""
                rows = P if g < G - 1 else LAST_ROWS
                h = act_pool.tile([P, D], f32, tag="hrow")
                nc.scalar.mul(h[:], z_ps[:], dis_sc[:, g:g + 1])
                nc.sync.dma_start(out=h_dram[g * P:g * P + rows, :],
                                  in_=h[:rows, :])

            # ---- prologue: table0 rows = dis * (x @ W0)
            for g in range(G):
                z = z_psum.tile([P, D], f32, space="PSUM")
                nc.tensor.matmul(out=z[:], lhsT=xT[:, g * P:(g + 1) * P],
                                 rhs=W[0][:], start=True, stop=True)
                own_rows(g, z, h_own)
            nc.gpsimd.collective_compute(
                "AllGather", mybir.AluOpType.bypass, replica_groups=RG,
                ins=[h_own.ap().opt()], outs=[tables[0].ap().opt()])

            # ---- 5 aggregation passes
            # pass i: act fn, next-layer weight index (None on last)
            passes = [("selu", 1), ("silu", 2), ("silu", 3),
                      ("softplus_neg", 4), ("final", None)][:n_passes]
            for pi, (fn, wnext) in enumerate(passes):
                tbl = tables[pi % 2]
                tbl_next = tables[(pi + 1) % 2]
                for g in range(G):
                    msg = msg_pool.tile([P, TT, D], f32)
                    nc.gpsimd.dma_gather(
                        msg[:, 0:TA, :], tbl[0:min(SPLIT, N), :],
                        idxA[:, g * TA * 8:(g + 1) * TA * 8],
                        TA * P, TA * P, D, single_packet=False)
                    if TB:
                        nc.gpsimd.dma_gather(
                            msg[:, TA:TT, :], tbl[SPLIT:N, :],
                            idxB[:, g * TB * 8:(g + 1) * TB * 8],
                            TB * P, TB * P, D, single_packet=False)
                    ps = agg_psum.tile([P, D], f32, space="PSUM")
                    for t in range(TT):
                        ind = ind_pool.tile([P, P], f32)
                        nc.vector.tensor_scalar(
                            out=ind[:], in0=iota[:],
                            scalar1=dstloc[:, g * TT + t:g * TT + t + 1],
                            scalar2=None, op0=mybir.AluOpType.is_equal)
                        nc.tensor.matmul(out=ps[:], lhsT=ind[:],
                                         rhs=msg[:, t, :],
                                         start=(t == 0), stop=(t == TT - 1))
                    # ---- evacuation: act = f(dis * ps + b)
                    act = act_pool.tile([P, D], f32, tag="act")
                    if use_bias:
                        lin = tmp_pool.tile([P, D], f32, tag="lin")
                        nc.vector.tensor_scalar(
                            out=lin[:], in0=ps[:],
                            scalar1=dis_sc[:, g:g + 1], scalar2=None,
                            op0=mybir.AluOpType.mult)
                        nc.vector.tensor_tensor(
                            out=lin[:], in0=lin[:], in1=BB[pi][:],
                            op=mybir.AluOpType.add)
                        srcx, s_sil, s_lam, s_neg = lin, 1.0, SELU_L, -1.0
                    else:
                        srcx = ps
                        s_sil = dis_sc[:, g:g + 1]
                        s_lam = dis_lam[:, g:g + 1]
                        s_neg = dis_neg[:, g:g + 1]
                    if fn == "silu":
                        if sim_safe:
                            sg = tmp_pool.tile([P, D], f32, tag="sg")
                            xx = tmp_pool.tile([P, D], f32, tag="xx")
                            nc.scalar.activation(sg[:], srcx[:], AF.Sigmoid,
                                                 scale=s_sil)
                            nc.scalar.mul(xx[:], srcx[:], s_sil)
                            nc.vector.tensor_tensor(
                                out=act[:], in0=sg[:], in1=xx[:],
                                op=mybir.AluOpType.mult)
                        else:
                            nc.scalar.activation(act[:], srcx[:], AF.Silu,
                                                 scale=s_sil)
                    elif fn == "softplus_neg":
                        # softplus(-x) = ln(1 + exp(-x))
                        e = tmp_pool.tile([P, D], f32, tag="sp_e")
                        nc.scalar.activation(e[:], srcx[:], AF.Exp,
                                             scale=s_neg)
                        nc.scalar.activation(act[:], e[:], AF.Ln, bias=1.0)
                    elif fn == "selu":
                        r = tmp_pool.tile([P, D], f32, tag="selu_r")
                        m = tmp_pool.tile([P, D], f32, tag="selu_m")
                        nc.scalar.activation(r[:], srcx[:], AF.Relu,
                                             scale=s_lam)
                        nc.scalar.activation(m[:], srcx[:], AF.Relu,
                                             scale=s_neg)
                        nc.scalar.activation(m[:], m[:], AF.Exp, scale=-1.0)
                        nc.vector.tensor_scalar(
                            out=m[:], in0=m[:],
                            scalar1=SELU_L * SELU_A, scalar2=-SELU_L * SELU_A,
                            op0=mybir.AluOpType.mult, op1=mybir.AluOpType.add)
                        nc.vector.tensor_tensor(out=act[:], in0=r[:],
                                                in1=m[:],
                                                op=mybir.AluOpType.add)
                    else:  # final
                        if use_bias:
                            nc.vector.tensor_copy(act[:], srcx[:])
                        else:
                            nc.scalar.mul(act[:], ps[:], dis_sc[:, g:g + 1])

                    rows = P if g < G - 1 else LAST_ROWS
                    if wnext is None:
                        nc.sync.dma_start(out=mu_out[g * P:g * P + rows, :],
                                          in_=act[:rows, 0:OUTC])
                        nc.sync.dma_start(out=lv_out[g * P:g * P + rows, :],
                                          in_=act[:rows, OUTC:D])
                    elif skip_own:
                        pass
                    else:
                        # own-rows stage for the next table
                        pT = tr_psum.tile([P, P], f32, space="PSUM")
                        nc.tensor.transpose(out=pT[:], in_=act[:],
                                            identity=ident[:])
                        hsT = tmp_pool.tile([P, P], f32, tag="hsT")
                        nc.vector.tensor_copy(hsT[:], pT[:])
                        z = z_psum.tile([P, D], f32, space="PSUM")
                        nc.tensor.matmul(out=z[:], lhsT=hsT[:],
                                         rhs=W[wnext][:],
                                         start=True, stop=True)
                        own_rows(g, z, h_own)
                if wnext is not None and not skip_own:
                    nc.gpsimd.collective_compute(
                        "AllGather", mybir.AluOpType.bypass, replica_groups=RG,
                        ins=[h_own.ap().opt()],
                        outs=[tbl_next.ap().opt()])
    nc.finalize()
    return nc


# ------------------------------------------------------------------- driver
def kernel(x, edge_index, W0, b0, W1, b1, W2, b2, W3, b3, Wmu, bmu, Wlv, blv):
    x = np.asarray(x, dtype=np.float32)
    edge_index = np.asarray(edge_index)
    assert x.shape == (N, D) and edge_index.shape == (2, E)

    TA, TB, per_core = _preprocess(edge_index)
    use_bias = any(np.any(np.asarray(b)) for b in (b0, b1, b2, b3, bmu, blv))

    key = (TA, TB, use_bias)
    if key not in _CACHE:
        _CACHE[key] = _build(TA, TB, use_bias)
    nc = _CACHE[key]

    iota = np.tile(np.arange(P, dtype=np.float32), (P, 1))
    ident = np.eye(P, dtype=np.float32)
    Wmulv = np.concatenate([-np.asarray(Wmu), -np.asarray(Wlv)],
                           axis=1).astype(np.float32)
    Ws = [np.asarray(w, dtype=np.float32) for w in (W0, W1, W2, W3)] + [Wmulv]

    in_maps = []
    for k in range(NCORES):
        pc = per_core[k]
        dis_k = pc["dis"]
        xT = np.zeros((P, G * P), np.float32)
        xs = x[k * NPC:(k + 1) * NPC]           # [6250, 128]
        xT[:, :NPC] = xs.T
        m = dict(idxA=pc["idxA"], dstloc=pc["dstloc"],
                 iota=iota, ident=ident, dis_sc=dis_k,
                 dis_lam=(SELU_L * dis_k).astype(np.float32),
                 dis_neg=(-dis_k).astype(np.float32), xT=xT)
        if TB:
            m["idxB"] = pc["idxB"]
        for i, w in enumerate(Ws):
            m[f"W{i}"] = w
        if use_bias:
            bmulv = np.concatenate([np.asarray(bmu), np.asarray(blv)])
            # bias of pass pi is conv pi's bias, broadcast across partitions
            for i, b in enumerate((b0, b1, b2, b3, bmulv)):
                bb = np.tile(np.asarray(b, dtype=np.float32)[None, :], (P, 1))
                m[f"BB{i}"] = bb.astype(np.float32)
        in_maps.append(m)

    res = run_bass_kernel_spmd(nc, in_maps, core_ids=list(range(NCORES)))
    mu = np.concatenate([res.results[k]["mu_out"] for k in range(NCORES)], axis=0)
    lv = np.concatenate([res.results[k]["lv_out"] for k in range(NCORES)], axis=0)
    return (mu, lv)
